# revision 16
# baseline (speedup 1.0000x reference)
"""Trainium2 Bass kernel for the two-template sparse cross-modal attention module.

Sharding: data-parallel over batch B=32 across 8 NeuronCores (4 samples/core).
Each sample carries two modality streams (v, i) that must be co-resident
because search tokens attend to the template keys of BOTH modalities.

Per-core program (per sample s, streams st in {v, i}):
  1. QK^T projection in transposed layout: QKT[1536, 384] = qkv_w[0:1536] @ x.T
     (lhsT = qkv_w.T chunks, rhs = x.T chunks) -> per-head Q.T, K.T [64, tok].
  2. V projection in natural layout: V[384, 768] = x @ qkv_w[1536:].T
     (lhsT = x.T chunks, rhs = qkv_w.T[:, 1536:]) stored with a ones column
     per head ([tok, 65]) so the AV matmul also accumulates the softmax
     denominator l as an extra output row.
  3. Attention per head, scores transposed (S.T[k, q] = K Q.T, contract Dh):
     softmax without max-subtraction (scores are O(1); exp is safe), the
     denominator comes from the ones column, normalization by 1/l applied via
     a gpsimd partition_broadcast of recip_l + one DVE multiply.
     Search queries attend to [own k_mt, other-modality k_mt, own k_s].
  4. Output projection from the transposed attention output (lhsT = O.T
     chunks, rhs = proj_w.T) -> natural-layout Y [384, 768], bias added via a
     K=1 ones matmul, contiguous DMA out.
"""

import numpy as np

for _p in ("/opt/trn_rl_repo", "/root/.axon_site/_ro/trn_rl_repo"):
    import os
    import sys

    if os.path.isdir(_p) and _p not in sys.path:
        sys.path.append(_p)

B = 32
N_CORES = 8
SAMPLES = 4  # per core
C = 768
NTOK = 384
H = 12
DH = 64
MT = 128  # template tokens
CCH = C // 128  # 6 contraction chunks
MCH = 12  # QK row chunks (1536/128)
TCH = NTOK // 128  # 3 token chunks
SCALE = DH ** (-0.5)

_PROG_CACHE = {}


def _build_program(mm_f32r, es_bf16, with_bias=True):
    import concourse.bass as bass  # noqa: F401
    import concourse.tile as tile
    from concourse import bacc, mybir

    f32 = mybir.dt.float32
    f32r = mybir.dt.float32r
    bf16 = mybir.dt.bfloat16
    mdt = f32r if mm_f32r else f32
    esdt = bf16 if es_bf16 else mdt
    Act = mybir.ActivationFunctionType

    nc = bacc.Bacc(None, target_bir_lowering=False)
    if mm_f32r or es_bf16:
        import contextlib

        _lp = nc.allow_low_precision(reason="fp32r/bf16 matmul inputs, fp32 PSUM accumulation")
    else:
        import contextlib

        _lp = contextlib.nullcontext()
    _lp.__enter__()

    xt_d = nc.dram_tensor("xt", [2 * SAMPLES, C, NTOK], f32, kind="ExternalInput")
    qkvw_d = nc.dram_tensor("qkvwT", [C, 3 * C], f32, kind="ExternalInput")
    projw_d = nc.dram_tensor("projwT", [C, C], f32, kind="ExternalInput")
    bias_d = nc.dram_tensor("bias", [1, C], f32, kind="ExternalInput")
    y_d = nc.dram_tensor("y", [2 * SAMPLES, NTOK, C], f32, kind="ExternalOutput")

    dma_in = nc.gpsimd if mm_f32r else nc.sync

    with tile.TileContext(nc) as tc:
        with (
            tc.tile_pool(name="consts", bufs=1) as consts,
            tc.tile_pool(name="xtp", bufs=2) as xtp,
            tc.tile_pool(name="qktp", bufs=1) as qktp,
            tc.tile_pool(name="v1p", bufs=1) as v1p,
            tc.tile_pool(name="otp", bufs=1) as otp,
            tc.tile_pool(name="esp", bufs=4) as esp,
            tc.tile_pool(name="rlp", bufs=2) as rlp,
            tc.tile_pool(name="rlbp", bufs=2) as rlbp,
            tc.tile_pool(name="yp", bufs=3) as yp,
            tc.tile_pool(name="pap", bufs=3, space="PSUM") as pap,
            tc.tile_pool(name="psp", bufs=3, space="PSUM") as psp,
            tc.tile_pool(name="pop", bufs=2, space="PSUM") as pop,
        ):
            qkvw_sb = consts.tile([128, CCH, 3 * C], mdt)
            projw_sb = consts.tile([128, CCH, C], mdt)
            bias_sb = consts.tile([1, C], mdt)
            ones_row = consts.tile([1, 128], mdt)
            ones_f32 = consts.tile([128, 128], f32)
            nc.vector.memset(ones_f32, 1.0)
            for c in range(CCH):
                dma_in.dma_start(
                    out=qkvw_sb[:, c, :], in_=qkvw_d[c * 128 : (c + 1) * 128, :]
                )
                dma_in.dma_start(
                    out=projw_sb[:, c, :], in_=projw_d[c * 128 : (c + 1) * 128, :]
                )
            dma_in.dma_start(out=bias_sb, in_=bias_d[:, :])
            nc.vector.tensor_copy(out=ones_row, in_=ones_f32[0:1, 0:128])

            for s in range(SAMPLES):
                xt_sb = xtp.tile([128, CCH, 2, NTOK], mdt, tag="xt")
                for st in range(2):
                    for c in range(CCH):
                        dma_in.dma_start(
                            out=xt_sb[:, c, st, :],
                            in_=xt_d[2 * s + st, c * 128 : (c + 1) * 128, :],
                        )

                # ---- phase 1: QK^T (transposed layout) ----
                qkt_sb = qktp.tile([128, MCH, 2, NTOK], mdt, tag="qkt")
                for m in range(MCH):
                    for st in range(2):
                        pq = pap.tile([128, NTOK], f32, tag="pa")
                        for c in range(CCH):
                            nc.tensor.matmul(
                                pq,
                                qkvw_sb[:, c, m * 128 : (m + 1) * 128],
                                xt_sb[:, c, st, :],
                                start=(c == 0),
                                stop=(c == CCH - 1),
                            )
                        nc.scalar.activation(
                            out=qkt_sb[:, m, st, :], in_=pq, func=Act.Copy
                        )

                # ---- phase 2: V (natural layout, with ones column) ----
                v1_sb = v1p.tile([128, TCH, 2, H, 65], mdt, tag="v1")
                for t in range(TCH):
                    for st in range(2):
                        for n in range(2):
                            pv = pap.tile([128, NTOK], f32, tag="pa")
                            for c in range(CCH):
                                nc.tensor.matmul(
                                    pv,
                                    xt_sb[:, c, st, t * 128 : (t + 1) * 128],
                                    qkvw_sb[:, c, 2 * C + n * NTOK : 2 * C + (n + 1) * NTOK],
                                    start=(c == 0),
                                    stop=(c == CCH - 1),
                                )
                            nc.vector.tensor_copy(
                                out=v1_sb[:, t, st, 6 * n : 6 * n + 6, 0:64],
                                in_=pv.rearrange("p (h d) -> p h d", h=6),
                            )
                nc.vector.tensor_copy(
                    out=v1_sb[:, :, :, :, 64:65],
                    in_=ones_f32[:, 0:72].rearrange(
                        "p (t s h) -> p t s h", t=TCH, s=2
                    ).unsqueeze(4),
                )

                # ---- phase 3: attention ----
                # Heads are processed in even/odd pairs: their Q.T/K.T slices
                # sit at partition bases 0 and 64, so the two K=64 score
                # matmuls target distinct PE row-groups; emitting them
                # back-to-back lets the hardware run them concurrently.
                ot_sb = otp.tile([128, CCH, 2, NTOK], mdt, tag="ot")
                for st in range(2):
                    for hp in range(6):
                        po_pair = [
                            pop.tile([65, NTOK], f32, tag="po", name=f"po_{s}_{st}_{hp}_{i}")
                            for i in range(2)
                        ]
                        # per chunk: S-mm pair (adjacent), exps, AV pair
                        for ci in range(4):
                            es_pair = []
                            ps_pair = []
                            for i in range(2):
                                h = 2 * hp + i
                                ro = i * 64
                                qT = qkt_sb[ro : ro + 64, hp, st, :]
                                kT = qkt_sb[ro : ro + 64, 6 + hp, st, :]
                                kTo = qkt_sb[ro : ro + 64, 6 + hp, 1 - st, :]
                                if ci == 0:
                                    lk, rq, nq = kT[:, 0:MT], qT, NTOK
                                elif ci == 1:
                                    lk, rq, nq = kTo[:, 0:MT], qT[:, MT:], 256
                                else:
                                    j = ci - 2
                                    lk = kT[:, MT + j * 128 : MT + (j + 1) * 128]
                                    rq, nq = qT[:, MT:], 256
                                psc = psp.tile(
                                    [128, nq], f32, tag="ps", name=f"ps_{s}_{st}_{hp}_{ci}_{i}"
                                )
                                nc.tensor.matmul(psc, lk, rq, start=True, stop=True)
                                ps_pair.append(psc)
                            for i in range(2):
                                ei = esp.tile(
                                    [128, nq], esdt, tag="es", name=f"es_{s}_{st}_{hp}_{ci}_{i}"
                                )
                                nc.scalar.activation(
                                    ei, ps_pair[i], Act.Exp, scale=SCALE
                                )
                                es_pair.append(ei)
                            for i in range(2):
                                h = 2 * hp + i
                                vst = (1 - st) if ci == 1 else st
                                vt = 0 if ci < 2 else ci - 1
                                dst = po_pair[i] if ci == 0 else po_pair[i][:, MT:]
                                nc.tensor.matmul(
                                    dst,
                                    v1_sb[:, vt, vst, h, :],
                                    es_pair[i],
                                    start=(ci == 0),
                                    stop=(ci == 3),
                                )
                        for i in range(2):
                            h = 2 * hp + i
                            ro = i * 64
                            po = po_pair[i]
                            rl = rlp.tile([1, NTOK], f32, tag="rl", name=f"rl_{s}_{st}_{hp}_{i}")
                            nc.vector.reciprocal(out=rl, in_=po[64:65, :])
                            rlb = rlbp.tile([64, NTOK], f32, tag="rlb", name=f"rlb_{s}_{st}_{hp}_{i}")
                            nc.gpsimd.partition_broadcast(rlb, rl)
                            nc.vector.tensor_mul(
                                ot_sb[ro : ro + 64, hp, st, :], po[0:64, :], rlb
                            )

                # ---- phase 4: output projection ----
                for st in range(2):
                    for t in range(TCH):
                        y_sb = yp.tile([128, C], f32, tag="y")
                        for n2 in range(2):
                            py = pap.tile([128, NTOK], f32, tag="pa")
                            for c in range(CCH):
                                nc.tensor.matmul(
                                    py,
                                    ot_sb[:, c, st, t * 128 : (t + 1) * 128],
                                    projw_sb[:, c, n2 * NTOK : (n2 + 1) * NTOK],
                                    start=(c == 0),
                                    stop=(not with_bias and c == CCH - 1),
                                )
                            if with_bias:
                                nc.tensor.matmul(
                                    py,
                                    ones_row[0:1, :],
                                    bias_sb[0:1, n2 * NTOK : (n2 + 1) * NTOK],
                                    start=False,
                                    stop=True,
                                )
                            nc.vector.tensor_copy(
                                out=y_sb[:, n2 * NTOK : (n2 + 1) * NTOK], in_=py
                            )
                        nc.sync.dma_start(
                            out=y_d[2 * s + st, t * 128 : (t + 1) * 128, :], in_=y_sb
                        )

    _lp.__exit__(None, None, None)
    nc.compile()
    return nc


def _get_program(mm_f32r=True, es_bf16=False, with_bias=True):
    key = (mm_f32r, es_bf16, with_bias)
    if key not in _PROG_CACHE:
        _PROG_CACHE[key] = _build_program(mm_f32r, es_bf16, with_bias)
    return _PROG_CACHE[key]


def _prep_in_maps(x_v, x_i, qkv_w, proj_w, proj_b):
    qkvwT = np.ascontiguousarray(qkv_w.T.astype(np.float32))
    projwT = np.ascontiguousarray(proj_w.T.astype(np.float32))
    bias = np.ascontiguousarray(proj_b.astype(np.float32).reshape(1, C))
    in_maps = []
    for core in range(N_CORES):
        sl = slice(core * SAMPLES, (core + 1) * SAMPLES)
        # interleave: stream 2s = v-sample, 2s+1 = i-sample, transposed to [C, NTOK]
        xs = np.empty((2 * SAMPLES, C, NTOK), np.float32)
        xs[0::2] = np.asarray(x_v[sl]).transpose(0, 2, 1)
        xs[1::2] = np.asarray(x_i[sl]).transpose(0, 2, 1)
        in_maps.append(
            {
                "xt": np.ascontiguousarray(xs),
                "qkvwT": qkvwT,
                "projwT": projwT,
                "bias": bias,
            }
        )
    return in_maps


def kernel(x_v, x_i, qkv_w, proj_w, proj_b, t_h, t_w, s_h, s_w, num_heads):
    from concourse.bass_utils import run_bass_kernel_spmd

    x_v = np.asarray(x_v, np.float32)
    x_i = np.asarray(x_i, np.float32)
    nc = _get_program(with_bias=bool(np.any(np.asarray(proj_b))))
    in_maps = _prep_in_maps(x_v, x_i, qkv_w, proj_w, proj_b)
    res = run_bass_kernel_spmd(nc, in_maps, list(range(N_CORES)))
    out_v = np.empty((B, NTOK, C), np.float32)
    out_i = np.empty((B, NTOK, C), np.float32)
    for core in range(N_CORES):
        y = res.results[core]["y"]
        sl = slice(core * SAMPLES, (core + 1) * SAMPLES)
        out_v[sl] = y[0::2]
        out_i[sl] = y[1::2]
    return out_v, out_i
